# revision 5
# baseline (speedup 1.0000x reference)
"""HardMoE (top-2 of 8 experts) on 8 Trainium2 NeuronCores, expert-parallel,
hybrid fp8/bf16 precision.

Strategy:
  - Host computes the small gate (x @ W_gate) in fp32 and the top-2 expert
    ids per token (set semantics match jax.lax.top_k since the output is a
    plain mean over the selected experts).
  - Tokens are dispatched host-side: core e receives the tokens routed to
    expert e (padded to a common capacity C so all cores run one SPMD
    program) plus expert e's weight matrix, pre-swizzled for the device
    layouts.
  - Each core computes relu(64*(X_e @ W_e) + 64*b_e) with a hybrid-precision
    contraction: the first 256*N_DR contraction columns run as fp8-e4m3
    DoubleRow matmuls (2 k-subtiles per matmul, ~2x tensor throughput), the
    remaining columns run bf16. fp32 PSUM accumulates across both parts in
    one accumulation group; relu+bias fused into the PSUM->SBUF eviction
    (alternating ScalarE/VectorE). W is pre-scaled by 64 so fp8 weight
    values are ~N(0,1), inside e4m3's normal range; since relu(a*z) =
    a*relu(z), the host combine divides by 64*TOP_K.
  - N_DR=4 (25% fp8) keeps the end-to-end max relative error at 1.83e-2
    (vs 2.17e-3 all-bf16, 3.62e-2 all-fp8) while cutting HW time ~21%.
  - Host gathers per-expert outputs and averages the two routed experts
    per token.
"""

import numpy as np
import ml_dtypes

import concourse.mybir as mybir
import concourse.tile as tile
from concourse import bacc
from concourse.bass_utils import run_bass_kernel_spmd

TOP_K = 2
E = 8
P = 128
D = 4096
H = 4096
KO = D // P
HT = H // P
F8 = ml_dtypes.float8_e4m3
BF16 = ml_dtypes.bfloat16
W_SCALE = 64.0
CAP_ROUND = 128
N_DR = 4  # fp8 DoubleRow pair-groups: covers 2*N_DR/32 = 25% of contraction


def _build_program(C: int, n_dr: int = N_DR, repeat: int = 1):
    KO8 = 2 * n_dr
    KOB = KO - KO8
    nc = bacc.Bacc("TRN2", target_bir_lowering=False, debug=False)
    xt8 = nc.dram_tensor("xt8", [P, KO8, C], mybir.dt.float8e4, kind="ExternalInput")
    xtb = nc.dram_tensor("xtb", [P, KOB, C], mybir.dt.bfloat16, kind="ExternalInput")
    w8 = nc.dram_tensor("w8", [HT, P, KO8, P], mybir.dt.float8e4, kind="ExternalInput")
    wb = nc.dram_tensor("wb", [HT, P, KOB, P], mybir.dt.bfloat16, kind="ExternalInput")
    b = nc.dram_tensor("b", [P, HT], mybir.dt.float32, kind="ExternalInput")
    yt = nc.dram_tensor("yt", [HT, P, C], mybir.dt.float32, kind="ExternalOutput")

    nchunks = -(-C // 512)
    chunks = [(i * 512, min(512, C - i * 512)) for i in range(nchunks)]

    # bf16 x is staged in NXG groups of subtiles (separate SBUF tiles) so the
    # next pass's reload of group g only waits for the last matmul touching
    # group g, overlapping the tail of the previous pass instead of
    # serializing the whole 13 MB load behind it. x8 double-buffers outright.
    NXG = 3
    assert KOB % NXG == 0
    KG = KOB // NXG

    with tile.TileContext(nc) as tc:
        with (
            tc.tile_pool(name="x8res", bufs=2) as x8pool,
            tc.tile_pool(name="xres", bufs=1) as xpool,
            tc.tile_pool(name="wstream", bufs=3) as wpool,
            tc.tile_pool(name="ostage", bufs=2) as opool,
            tc.tile_pool(name="const", bufs=1) as cpool,
            tc.tile_pool(name="psacc", bufs=8, space="PSUM") as pspool,
        ):
            for rep in range(repeat):
                bsb = cpool.tile([P, HT], mybir.dt.float32, tag="b")
                nc.scalar.dma_start(out=bsb[:], in_=b[:])
                x8sb = x8pool.tile([P, KO8, C], mybir.dt.float8e4, tag="x8")
                xbsb = [
                    xpool.tile(
                        [P, KG, C],
                        mybir.dt.bfloat16,
                        tag=f"xb{g}",
                        name=f"xb{g}_{rep}",
                    )
                    for g in range(NXG)
                ]
                for k in range(KO8):
                    nc.sync.dma_start(out=x8sb[:, k, :], in_=xt8[:, k, :])
                for k in range(KOB):
                    nc.sync.dma_start(
                        out=xbsb[k // KG][:, k % KG, :], in_=xtb[:, k, :]
                    )
                for ht in range(HT):
                    w8sb = wpool.tile([P, KO8, P], mybir.dt.float8e4, tag="w8")
                    wbsb = wpool.tile([P, KOB, P], mybir.dt.bfloat16, tag="wb")
                    # W strips on the ACT HWDGE ring so the first strips are
                    # not queued behind the X-slice DMAs on the sync ring.
                    nc.scalar.dma_start(out=w8sb[:], in_=w8[ht])
                    nc.scalar.dma_start(out=wbsb[:], in_=wb[ht])
                    pts = [
                        pspool.tile(
                            [P, cw],
                            mybir.dt.float32,
                            tag="ps",
                            name=f"ps{rep}_{ht}_{ci}",
                        )
                        for ci, (off, cw) in enumerate(chunks)
                    ]
                    for kk in range(n_dr):
                        lw = w8sb[:, 2 * kk : 2 * kk + 2, :]
                        for ci, (off, cw) in enumerate(chunks):
                            nc.tensor.matmul(
                                pts[ci][:],
                                lhsT=lw,
                                rhs=x8sb[:, 2 * kk : 2 * kk + 2, off : off + cw],
                                start=(kk == 0),
                                stop=False,
                                perf_mode=mybir.MatmulPerfMode.DoubleRow,
                            )
                    for k in range(KOB):
                        lw = wbsb[:, k, :]
                        for ci, (off, cw) in enumerate(chunks):
                            nc.tensor.matmul(
                                pts[ci][:],
                                lhsT=lw,
                                rhs=xbsb[k // KG][:, k % KG, off : off + cw],
                                start=False,
                                stop=(k == KOB - 1),
                            )
                    osb = opool.tile([P, C], mybir.dt.float32, tag="o")
                    bias = bsb[:, ht : ht + 1]
                    for ci, (off, cw) in enumerate(chunks):
                        # relu(z + 64*b[h]); per-partition bias rides the
                        # PSUM->SBUF eviction on ACT/DVE for free.
                        if ci % 2 == 0:
                            nc.scalar.activation(
                                osb[:, off : off + cw],
                                pts[ci][:],
                                mybir.ActivationFunctionType.Relu,
                                bias=bias,
                            )
                        else:
                            nc.vector.tensor_scalar(
                                osb[:, off : off + cw],
                                pts[ci][:],
                                bias,
                                0.0,
                                mybir.AluOpType.add,
                                mybir.AluOpType.max,
                            )
                    nc.sync.dma_start(out=yt[ht], in_=osb[:])
    nc.compile()
    return nc


def _prepare(x, W_gate, b_gate, W_e, b_e, n_dr=N_DR, cap_round=CAP_ROUND):
    """Gate + routing + per-core hybrid-precision input construction."""
    B, S, Dx = x.shape
    assert Dx == D
    KO8 = 2 * n_dr
    D8 = KO8 * P
    xf = np.ascontiguousarray(x.reshape(-1, D), dtype=np.float32)
    T = xf.shape[0]

    logits = xf @ np.asarray(W_gate, dtype=np.float32)
    logits += np.asarray(b_gate, dtype=np.float32)
    top2 = np.argsort(-logits, axis=1, kind="stable")[:, :TOP_K]

    ids = [np.nonzero((top2 == e).any(axis=1))[0] for e in range(E)]
    counts = np.array([len(i) for i in ids])
    C = max(cap_round, int(-(-counts.max() // cap_round)) * cap_round)

    x8 = xf[:, :D8].astype(F8)
    xb = xf[:, D8:].astype(BF16)
    in_maps = []
    for e in range(E):
        pad = np.zeros(C, dtype=np.int64)
        pad[: counts[e]] = ids[e]
        xt8 = x8[pad].reshape(C, KO8, P).transpose(2, 1, 0)
        xtb = xb[pad].reshape(C, KO - KO8, P).transpose(2, 1, 0)
        wsc = np.asarray(W_e[e], dtype=np.float32) * W_SCALE
        w8 = wsc[:D8].astype(F8).reshape(KO8, P, HT, P).transpose(2, 1, 0, 3)
        wb = wsc[D8:].astype(BF16).reshape(KO - KO8, P, HT, P).transpose(2, 1, 0, 3)
        bsw = np.ascontiguousarray(
            (np.asarray(b_e[e], dtype=np.float32) * W_SCALE).reshape(HT, P).T
        )
        in_maps.append(
            {
                "xt8": np.ascontiguousarray(xt8),
                "xtb": np.ascontiguousarray(xtb),
                "w8": np.ascontiguousarray(w8),
                "wb": np.ascontiguousarray(wb),
                "b": bsw,
            }
        )

    meta = dict(B=B, S=S, T=T, C=C, ids=ids, counts=counts, n_dr=n_dr)
    return in_maps, meta


def _combine(results, meta):
    """Average the two routed experts per token; undo the 64x W scale."""
    T, C = meta["T"], meta["C"]
    out = np.zeros((T, H), dtype=np.float32)
    for e in range(E):
        yt = np.asarray(results[e]["yt"], dtype=np.float32).reshape(H, C)
        out[meta["ids"][e]] += yt[:, : meta["counts"][e]].T
    out *= 1.0 / (TOP_K * W_SCALE)
    return out.reshape(meta["B"], meta["S"], H)


def kernel(x, W_gate, b_gate, W_e, b_e):
    in_maps, meta = _prepare(x, W_gate, b_gate, W_e, b_e)
    nc = _build_program(meta["C"])
    res = run_bass_kernel_spmd(nc, in_maps, list(range(E)))
    return _combine(res.results, meta)


# revision 7
# speedup vs baseline: 1.1280x; 1.1280x over previous
"""HardMoE (top-2 of 8 experts) on 8 Trainium2 NeuronCores, expert-parallel,
hybrid fp8/bf16 precision.

Strategy:
  - Host computes the small gate (x @ W_gate) in fp32 and the top-2 expert
    ids per token (set semantics match jax.lax.top_k since the output is a
    plain mean over the selected experts).
  - Tokens are dispatched host-side: core e receives the tokens routed to
    expert e (padded to a common capacity C so all cores run one SPMD
    program) plus expert e's weight matrix, pre-swizzled for the device
    layouts.
  - Each core computes relu(64*(X_e @ W_e) + 64*b_e) with a hybrid-precision
    contraction: the first 256*N_DR contraction columns run as fp8-e4m3
    DoubleRow matmuls (2 k-subtiles per matmul, ~2x tensor throughput), the
    remaining columns run bf16. fp32 PSUM accumulates across both parts in
    one accumulation group; relu+bias fused into the PSUM->SBUF eviction
    (alternating ScalarE/VectorE). W is pre-scaled by 64 so fp8 weight
    values are ~N(0,1), inside e4m3's normal range; since relu(a*z) =
    a*relu(z), the host combine divides by 64*TOP_K.
  - N_DR=4 (25% fp8) keeps the end-to-end max relative error at 1.83e-2
    (vs 2.17e-3 all-bf16, 3.62e-2 all-fp8) while cutting HW time ~21%.
  - Host gathers per-expert outputs and averages the two routed experts
    per token.
"""

import numpy as np
import ml_dtypes

import concourse.mybir as mybir
import concourse.tile as tile
from concourse import bacc
from concourse.bass_utils import run_bass_kernel_spmd

TOP_K = 2
E = 8
P = 128
D = 4096
H = 4096
KO = D // P
HT = H // P
F8 = ml_dtypes.float8_e4m3
BF16 = ml_dtypes.bfloat16
W_SCALE = 64.0
CAP_ROUND = 128
N_DR = 4  # fp8 DoubleRow pair-groups: covers 2*N_DR/32 = 25% of contraction


def _build_program(C: int, n_dr: int = N_DR, repeat: int = 1):
    KO8 = 2 * n_dr
    KOB = KO - KO8
    nc = bacc.Bacc("TRN2", target_bir_lowering=False, debug=False)
    xt8 = nc.dram_tensor("xt8", [P, KO8, C], mybir.dt.float8e4, kind="ExternalInput")
    xtb = nc.dram_tensor("xtb", [P, KOB, C], mybir.dt.bfloat16, kind="ExternalInput")
    w8 = nc.dram_tensor("w8", [HT, P, KO8, P], mybir.dt.float8e4, kind="ExternalInput")
    wb = nc.dram_tensor("wb", [HT, P, KOB, P], mybir.dt.bfloat16, kind="ExternalInput")
    b = nc.dram_tensor("b", [P, HT], mybir.dt.float32, kind="ExternalInput")
    yt = nc.dram_tensor("yt", [HT, P, C], mybir.dt.float32, kind="ExternalOutput")

    nchunks = -(-C // 512)
    chunks = [(i * 512, min(512, C - i * 512)) for i in range(nchunks)]

    with tile.TileContext(nc) as tc:
        with (
            tc.tile_pool(name="xres", bufs=1) as xpool,
            tc.tile_pool(name="wstream", bufs=3) as wpool,
            tc.tile_pool(name="ostage", bufs=2) as opool,
            tc.tile_pool(name="const", bufs=1) as cpool,
            tc.tile_pool(name="psacc", bufs=8, space="PSUM") as pspool,
        ):
            for rep in range(repeat):
                bsb = cpool.tile([P, HT], mybir.dt.float32, tag="b")
                nc.scalar.dma_start(out=bsb[:], in_=b[:])
                x8sb = xpool.tile([P, KO8, C], mybir.dt.float8e4, tag="x8")
                xbsb = xpool.tile([P, KOB, C], mybir.dt.bfloat16, tag="xb")
                for k in range(KO8):
                    nc.sync.dma_start(out=x8sb[:, k, :], in_=xt8[:, k, :])
                for k in range(KOB):
                    nc.sync.dma_start(out=xbsb[:, k, :], in_=xtb[:, k, :])
                for ht in range(HT):
                    w8sb = wpool.tile([P, KO8, P], mybir.dt.float8e4, tag="w8")
                    wbsb = wpool.tile([P, KOB, P], mybir.dt.bfloat16, tag="wb")
                    # W strips on the ACT HWDGE ring so the first strips are
                    # not queued behind the X-slice DMAs on the sync ring.
                    nc.scalar.dma_start(out=w8sb[:], in_=w8[ht])
                    nc.scalar.dma_start(out=wbsb[:], in_=wb[ht])
                    pts = [
                        pspool.tile(
                            [P, cw],
                            mybir.dt.float32,
                            tag="ps",
                            name=f"ps{rep}_{ht}_{ci}",
                        )
                        for ci, (off, cw) in enumerate(chunks)
                    ]
                    for kk in range(n_dr):
                        lw = w8sb[:, 2 * kk : 2 * kk + 2, :]
                        for ci, (off, cw) in enumerate(chunks):
                            nc.tensor.matmul(
                                pts[ci][:],
                                lhsT=lw,
                                rhs=x8sb[:, 2 * kk : 2 * kk + 2, off : off + cw],
                                start=(kk == 0),
                                stop=False,
                                perf_mode=mybir.MatmulPerfMode.DoubleRow,
                            )
                    for k in range(KOB):
                        lw = wbsb[:, k, :]
                        for ci, (off, cw) in enumerate(chunks):
                            nc.tensor.matmul(
                                pts[ci][:],
                                lhsT=lw,
                                rhs=xbsb[:, k, off : off + cw],
                                start=False,
                                stop=(k == KOB - 1),
                            )
                    osb = opool.tile([P, C], mybir.dt.float32, tag="o")
                    bias = bsb[:, ht : ht + 1]
                    for ci, (off, cw) in enumerate(chunks):
                        # relu(z + 64*b[h]); per-partition bias rides the
                        # PSUM->SBUF eviction on ACT/DVE for free.
                        if ci % 2 == 0:
                            nc.scalar.activation(
                                osb[:, off : off + cw],
                                pts[ci][:],
                                mybir.ActivationFunctionType.Relu,
                                bias=bias,
                            )
                        else:
                            nc.vector.tensor_scalar(
                                osb[:, off : off + cw],
                                pts[ci][:],
                                bias,
                                0.0,
                                mybir.AluOpType.add,
                                mybir.AluOpType.max,
                            )
                    nc.sync.dma_start(out=yt[ht], in_=osb[:])
    nc.compile()
    return nc


def _prepare(x, W_gate, b_gate, W_e, b_e, n_dr=N_DR, cap_round=CAP_ROUND):
    """Gate + routing + per-core hybrid-precision input construction."""
    B, S, Dx = x.shape
    assert Dx == D
    KO8 = 2 * n_dr
    D8 = KO8 * P
    xf = np.ascontiguousarray(x.reshape(-1, D), dtype=np.float32)
    T = xf.shape[0]

    logits = xf @ np.asarray(W_gate, dtype=np.float32)
    logits += np.asarray(b_gate, dtype=np.float32)
    top2 = np.argsort(-logits, axis=1, kind="stable")[:, :TOP_K]

    ids = [np.nonzero((top2 == e).any(axis=1))[0] for e in range(E)]
    counts = np.array([len(i) for i in ids])
    C = max(cap_round, int(-(-counts.max() // cap_round)) * cap_round)

    x8 = xf[:, :D8].astype(F8)
    xb = xf[:, D8:].astype(BF16)
    in_maps = []
    for e in range(E):
        pad = np.zeros(C, dtype=np.int64)
        pad[: counts[e]] = ids[e]
        xt8 = x8[pad].reshape(C, KO8, P).transpose(2, 1, 0)
        xtb = xb[pad].reshape(C, KO - KO8, P).transpose(2, 1, 0)
        wsc = np.asarray(W_e[e], dtype=np.float32) * W_SCALE
        w8 = wsc[:D8].astype(F8).reshape(KO8, P, HT, P).transpose(2, 1, 0, 3)
        wb = wsc[D8:].astype(BF16).reshape(KO - KO8, P, HT, P).transpose(2, 1, 0, 3)
        bsw = np.ascontiguousarray(
            (np.asarray(b_e[e], dtype=np.float32) * W_SCALE).reshape(HT, P).T
        )
        in_maps.append(
            {
                "xt8": np.ascontiguousarray(xt8),
                "xtb": np.ascontiguousarray(xtb),
                "w8": np.ascontiguousarray(w8),
                "wb": np.ascontiguousarray(wb),
                "b": bsw,
            }
        )

    meta = dict(B=B, S=S, T=T, C=C, ids=ids, counts=counts, n_dr=n_dr)
    return in_maps, meta


def _combine(results, meta):
    """Average the two routed experts per token; undo the 64x W scale."""
    T, C = meta["T"], meta["C"]
    out = np.zeros((T, H), dtype=np.float32)
    for e in range(E):
        yt = np.asarray(results[e]["yt"], dtype=np.float32).reshape(H, C)
        out[meta["ids"][e]] += yt[:, : meta["counts"][e]].T
    out *= 1.0 / (TOP_K * W_SCALE)
    return out.reshape(meta["B"], meta["S"], H)


def kernel(x, W_gate, b_gate, W_e, b_e):
    in_maps, meta = _prepare(x, W_gate, b_gate, W_e, b_e)
    nc = _build_program(meta["C"])
    res = run_bass_kernel_spmd(nc, in_maps, list(range(E)))
    return _combine(res.results, meta)
